# revision 1
# baseline (speedup 1.0000x reference)
"""AnyprecisionLinear (w_bits=4) on 8 TRN2 NeuronCores — self-contained kernel.

kernel(x, qweight, lut) -> out
  x       (1, 2048, 8192) f32
  qweight (8192, 2048)    int32   (4x 8-bit codes per word; idx = code >> 4)
  lut     (8192, 16)      f32     (per-output-row 16-entry table)
  out     (1, 2048, 8192) f32     == einsum('bsk,ok->bso', x, lut[o, idx[o,k]])

Sharding: column-parallel — core i owns output rows [1024*i, 1024*(i+1)).
Each core gets its qweight/lut shard, the full x, and computes out^T (o, s);
the host concatenates the 8 (1024, 2048) shards and transposes.

Per-core kernel:
  - LUT rows are bf16-packed on the host into pair words
    T_m[o] = bits(lut[o,2m]) | bits(lut[o,2m+1])<<16  (m = idx>>1), shipped as
    tb_t = T_{2t}, dl_t = T_{2t}^T_{2t+1} (t = 0..3).
  - Dequant on DVE, o-rows on partitions, per (128,1024) chunk of code bytes c:
      b1 mask M1 (full-lane)  : t1 = (c32>>5)&0x01010101 ; M1 = t1_u8 * -1 -> s32 (Pool)
      z_t = (M1 & dl_t) ^ tb_t           4x u32 tensor_scalar, per-partition AP scalars
      z0<-z1 (b2) ; z2<-z3 (b2) ; z0<-z2 (b3)     copy_predicated, u8 bit masks
      lo16(z0) <- hi16(z0) where b0                strided copy_predicated
    leaving bf16 weights at even u16 positions of z0.
  - W chunks are PE-transposed (128x128, strided bf16 reads) into resident
    Wt (k, o) tiles; ScalarE drains the PSUM.
  - x is cast f32->bf16 by DRAM->DRAM converting DMAs (SWDGE), then
    DMA-transposed (hardware xbar) into (k, s) tiles on demand.
  - bf16 matmuls accumulate out^T in PSUM over 64 k-tiles; two o-tile groups
    of 4 reuse each (512-token, 128-k) xt tile; ScalarE drains accumulators.
"""
import numpy as np
import ml_dtypes

import concourse.mybir as mybir
from concourse import bacc, bass_utils
from concourse.tile import TileContext
from concourse.masks import make_identity

dt = mybir.dt
A = mybir.AluOpType

O, K, S = 1024, 8192, 2048     # per-core out rows, contraction, tokens
P = 128
OT = O // P
KT = K // P
KC = 1024                      # dequant k-chunk
NCH = K // KC
SBW = 512                      # tokens per psum accumulator
NSB = S // SBW
GROUPS = [(0, 4), (4, 4)]      # (first o_tile, count) per matmul group
N_CORES = 8


def _host_tables(lut_shard):
    U = lut_shard.astype(ml_dtypes.bfloat16).view(np.uint16).astype(np.uint32)
    T = U[:, 0::2] | (U[:, 1::2] << 16)
    tb = T[:, 0::2].copy()
    dl = (T[:, 0::2] ^ T[:, 1::2]).copy()
    return tb, dl


def _build_kernel(nc):
    x_in = nc.declare_dram_parameter("x", [S, K], dt.float32, isOutput=False)
    qw_in = nc.declare_dram_parameter("qw", [O, K], dt.uint8, isOutput=False)
    tb_in = nc.declare_dram_parameter("tb", [O, 4], dt.uint32, isOutput=False)
    dl_in = nc.declare_dram_parameter("dl", [O, 4], dt.uint32, isOutput=False)
    out_d = nc.declare_dram_parameter("out", [O, S], dt.float32, isOutput=True)
    xbf_d = nc.dram_tensor("xbf", [S, K], dt.bfloat16)

    with TileContext(nc) as tc:
        with tc.tile_pool(name="const", bufs=1) as cpool, \
             tc.tile_pool(name="wt", bufs=1) as wtpool, \
             tc.tile_pool(name="tabs", bufs=1) as tabpool, \
             tc.tile_pool(name="deq", bufs=1) as dqpool, \
             tc.tile_pool(name="xt", bufs=8) as xtpool, \
             tc.tile_pool(name="outd", bufs=3) as outpool, \
             tc.tile_pool(name="pst", bufs=2, space="PSUM") as pst, \
             tc.tile_pool(name="psacc", bufs=1, space="PSUM") as psacc:

            ident = cpool.tile([P, P], dt.bfloat16, name="ident")
            idf = cpool.tile([P, P], dt.float32, name="idf")
            make_identity(nc, idf)
            nc.vector.tensor_copy(out=ident, in_=idf)

            tabs = []
            for ot in range(OT):
                tb_sb = tabpool.tile([P, 4], dt.uint32, name=f"tb{ot}")
                dl_sb = tabpool.tile([P, 4], dt.uint32, name=f"dl{ot}")
                nc.sync.dma_start(out=tb_sb, in_=tb_in[ot * P:(ot + 1) * P, :])
                nc.sync.dma_start(out=dl_sb, in_=dl_in[ot * P:(ot + 1) * P, :])
                tabs.append((tb_sb, dl_sb))

            wts = [wtpool.tile([P, K], dt.bfloat16, name=f"wt{ot}") for ot in range(OT)]

            def xcast_phase():
                for sc in range(S // P):
                    nc.gpsimd.dma_start(out=xbf_d[sc * P:(sc + 1) * P, :],
                                        in_=x_in[sc * P:(sc + 1) * P, :])

            def dequant_otile(ot):
                tb_sb, dl_sb = tabs[ot]
                qw8 = dqpool.tile([P, K], dt.uint8, name="qw8", tag="qw8", bufs=1)
                nc.sync.dma_start(out=qw8, in_=qw_in[ot * P:(ot + 1) * P, :])
                for ch in range(NCH):
                    c8 = qw8[:, ch * KC:(ch + 1) * KC]
                    cw = c8.bitcast(dt.uint32)
                    t1 = dqpool.tile([P, KC // 4], dt.uint32, name="t1", tag="t1", bufs=2)
                    nc.vector.tensor_scalar(out=t1, in0=cw, scalar1=5, scalar2=0x01010101,
                                            op0=A.logical_shift_right, op1=A.bitwise_and)
                    m2 = dqpool.tile([P, KC // 4], dt.uint32, name="m2", tag="m2", bufs=2)
                    nc.vector.tensor_scalar(out=m2, in0=cw, scalar1=0x40404040, scalar2=None,
                                            op0=A.bitwise_and)
                    m3 = dqpool.tile([P, KC], dt.uint8, name="m3", tag="m3", bufs=2)
                    nc.gpsimd.tensor_scalar(out=m3, in0=c8, scalar1=128.0, scalar2=None,
                                            op0=A.is_ge)
                    b0 = dqpool.tile([P, KC // 4], dt.uint32, name="b0", tag="b0", bufs=2)
                    nc.vector.tensor_scalar(out=b0, in0=cw, scalar1=0x10101010, scalar2=None,
                                            op0=A.bitwise_and)

                    m1 = dqpool.tile([P, KC], dt.int32, name="m1", tag="m1", bufs=1)
                    nc.gpsimd.tensor_scalar(out=m1, in0=t1.bitcast(dt.uint8), scalar1=-1.0,
                                            scalar2=None, op0=A.mult)

                    zs = []
                    for t in range(4):
                        z = dqpool.tile([P, KC], dt.uint32, name=f"z{t}", tag=f"z{t}", bufs=2)
                        nc.vector.tensor_scalar(out=z, in0=m1.bitcast(dt.uint32),
                                                scalar1=dl_sb[:, t:t + 1], scalar2=tb_sb[:, t:t + 1],
                                                op0=A.bitwise_and, op1=A.bitwise_xor)
                        zs.append(z)
                    nc.vector.copy_predicated(out=zs[0], mask=m2.bitcast(dt.uint8), data=zs[1])
                    nc.vector.copy_predicated(out=zs[2], mask=m2.bitcast(dt.uint8), data=zs[3])
                    nc.vector.copy_predicated(out=zs[0], mask=m3, data=zs[2])

                    zv = zs[0].bitcast(dt.uint16).rearrange("p (k two) -> p k two", two=2)
                    nc.vector.copy_predicated(out=zv[:, :, 0], mask=b0.bitcast(dt.uint8), data=zv[:, :, 1])

                    wch = zs[0].bitcast(dt.bfloat16).rearrange("p (k two) -> p k two", two=2)[:, :, 0]
                    for jg in range(KC // P // 4):
                        pt = pst.tile([P, 4 * P], dt.bfloat16, name="pt", tag="pt")
                        for j4 in range(4):
                            j = jg * 4 + j4
                            nc.tensor.transpose(pt[:, j4 * P:(j4 + 1) * P],
                                                wch[:, j * P:(j + 1) * P], ident)
                        kt0 = ch * (KC // P) + jg * 4
                        nc.scalar.copy(out=wts[ot][:, kt0 * P:(kt0 + 4) * P], in_=pt)

            def matmul_group(g0, gn):
                for sb in range(NSB):
                    accs = [psacc.tile([P, SBW], dt.float32, name=f"acc{g0}_{sb}_{i}", tag=f"acc{i}")
                            for i in range(gn)]
                    for kt in range(KT):
                        xt = xtpool.tile([P, SBW], dt.bfloat16, name="xt", tag="xt")
                        nc.sync.dma_start_transpose(
                            out=xt, in_=xbf_d[sb * SBW:(sb + 1) * SBW, kt * P:(kt + 1) * P])
                        for i in range(gn):
                            ot = g0 + i
                            nc.tensor.matmul(accs[i], wts[ot][:, kt * P:(kt + 1) * P], xt,
                                             start=(kt == 0), stop=(kt == KT - 1))
                    for i in range(gn):
                        ot = g0 + i
                        ob = outpool.tile([P, SBW], dt.float32, name="ob", tag="ob")
                        nc.scalar.copy(out=ob, in_=accs[i])
                        nc.sync.dma_start(
                            out=out_d[ot * P:(ot + 1) * P, sb * SBW:(sb + 1) * SBW], in_=ob)

            for i in range(4):
                dequant_otile(i)
            xcast_phase()
            for i in range(4):
                dequant_otile(4 + i)
            for (g0, gn) in GROUPS:
                matmul_group(g0, gn)


_NC_CACHE = None


def _get_nc():
    global _NC_CACHE
    if _NC_CACHE is None:
        nc = bacc.Bacc("TRN2", num_devices=N_CORES)
        _build_kernel(nc)
        nc.compile()
        _NC_CACHE = nc
    return _NC_CACHE


def kernel(x, qweight, lut):
    x = np.asarray(x)
    qweight = np.asarray(qweight)
    lut = np.asarray(lut)
    assert x.shape == (1, S, K) and qweight.shape == (K, S // 4 * 4 // 4) or True
    x2 = np.ascontiguousarray(x.reshape(S, K).astype(np.float32, copy=False))

    in_maps = []
    for c in range(N_CORES):
        o0, o1 = c * O, (c + 1) * O
        qb = np.ascontiguousarray(qweight[o0:o1]).view(np.uint8).reshape(O, K)
        tb, dl = _host_tables(lut[o0:o1])
        in_maps.append({"x": x2, "qw": qb, "tb": tb, "dl": dl})

    nc = _get_nc()
    res = bass_utils.run_bass_kernel_spmd(nc, in_maps, core_ids=list(range(N_CORES)))
    out_full = np.concatenate([res.results[c]["out"] for c in range(N_CORES)], axis=0)
    return np.ascontiguousarray(out_full.T).reshape(1, S, 8192).astype(np.float32, copy=False)



# revision 15
# speedup vs baseline: 1.2036x; 1.2036x over previous
"""AnyprecisionLinear (w_bits=4) on 8 TRN2 NeuronCores — self-contained kernel.

kernel(x, qweight, lut) -> out
  x       (1, 2048, 8192) f32
  qweight (8192, 2048)    int32   (4x 8-bit codes per word; idx = code >> 4)
  lut     (8192, 16)      f32     (per-output-row 16-entry table)
  out     (1, 2048, 8192) f32     == einsum('bsk,ok->bso', x, lut[o, idx[o,k]])

Sharding: column-parallel — core i owns output rows [1024*i, 1024*(i+1)).
Each core gets its qweight/lut shard plus the full x (host-cast to bf16),
computes out^T (o, s); the host concatenates the 8 (1024, 2048) shards and
transposes.

Per-core kernel = software-pipelined dequant+matmul:
  - k is split into 4 round-chunks of 2048 per otile; rounds r=0..7 cover
    (group, chunk) pairs for 2 groups of 4 otiles.
  - Dequant (round r+1) is issued interleaved into round r's matmul passes so
    DVE/Pool dequant hides under PE matmul work.
  - Dequant per chunk: byte codes -> per-code full-lane mask m1 (DVE),
    z_t = (m1 & dl_t) ^ tb_t (4x DVE tensor_scalar, bf16-pair tables),
    pair-tree merges via copy_predicated: z2<-z3 on DVE, z0<-z1, z0<-z2 and
    the final u16 lo/hi select on Pool (mask bytes from cheap DVE
    tensor_scalars: &0x40.., &0x80.., &0x10..).
  - Weights are PE-transposed (128x128) into per-(slot, round) Wt tiles.
  - Matmuls accumulate 16-ktile segments per (otile, 512-token sb) in PSUM;
    segments are combined across rounds in SBUF f32 accumulators (Act copy on
    round 0, DVE/Pool scalar_tensor_tensor adds on later rounds).
  - x^T tiles stream via hardware transpose-DMAs from the host-cast bf16 x.
"""
import numpy as np
import ml_dtypes

import concourse.mybir as mybir
from concourse import bacc, bass_utils
from concourse.tile import TileContext
from concourse.masks import make_identity

dt = mybir.dt
A = mybir.AluOpType

O, K, S = 1024, 8192, 2048    # per-core out rows, contraction, tokens
P = 128
KC = 2048                     # codes per dequant chunk (k span per round)
NCH = K // KC                 # rounds (chunk index) per otile = 4
SEG = KC // P                 # ktiles per segment = 16
SBW = 512                     # tokens per psum segment accumulator
NSB = S // SBW                # 4 sb passes per round
GN = 4                        # otiles per group
NG = O // P // GN             # 2 groups
NR = NCH * NG                 # 8 global rounds
N_CORES = 8


def _host_tables(lut_shard):
    U = lut_shard.astype(ml_dtypes.bfloat16).view(np.uint16).astype(np.uint32)
    T = U[:, 0::2] | (U[:, 1::2] << 16)
    tb = T[:, 0::2].copy()
    dl = (T[:, 0::2] ^ T[:, 1::2]).copy()
    return tb, dl


def _build_kernel(nc):
    xbf_in = nc.declare_dram_parameter("xbf", [S, K], dt.bfloat16, isOutput=False)
    qw_in = nc.declare_dram_parameter("qw", [O, K], dt.uint8, isOutput=False)
    tb_in = nc.declare_dram_parameter("tb", [O, 4], dt.uint32, isOutput=False)
    dl_in = nc.declare_dram_parameter("dl", [O, 4], dt.uint32, isOutput=False)
    out_d = nc.declare_dram_parameter("out", [O, S], dt.float32, isOutput=True)

    with TileContext(nc) as tc:
        with tc.tile_pool(name="const", bufs=1) as cpool, \
             tc.tile_pool(name="tabs", bufs=1) as tabpool, \
             tc.tile_pool(name="wt", bufs=1) as wtpool, \
             tc.tile_pool(name="sacc", bufs=1) as saccpool, \
             tc.tile_pool(name="deq", bufs=1) as dqpool, \
             tc.tile_pool(name="xt", bufs=1) as xtpool, \
             tc.tile_pool(name="pst", bufs=1, space="PSUM") as pst, \
             tc.tile_pool(name="psacc", bufs=1, space="PSUM") as psacc:

            ident = cpool.tile([P, P], dt.bfloat16, name="ident")
            idf = cpool.tile([P, P], dt.float32, name="idf")
            make_identity(nc, idf)
            nc.vector.tensor_copy(out=ident, in_=idf)

            NOT = O // P
            tb_all = tabpool.tile([P, 4 * NOT], dt.uint32, name="tb_all")
            dl_all = tabpool.tile([P, 4 * NOT], dt.uint32, name="dl_all")
            nc.sync.dma_start(out=tb_all.rearrange("p (g t) -> p g t", t=4),
                              in_=tb_in.rearrange("(g p) t -> p g t", p=P))
            nc.sync.dma_start(out=dl_all.rearrange("p (g t) -> p g t", t=4),
                              in_=dl_in.rearrange("(g p) t -> p g t", p=P))
            tabs = [(tb_all[:, 4 * ot:4 * (ot + 1)], dl_all[:, 4 * ot:4 * (ot + 1)])
                    for ot in range(NOT)]

            wts = {}          # (slot, c) -> Wt tile [P, KC] (k on partitions)
            pending = {}      # slot -> (z0, c) awaiting transpose
            xt_tiles = {}     # (sb, j) -> xt tile [P, SBW]
            saccs = {}        # slot -> SBUF accumulator [P, S]

            def deq_compute(r, slot):
                g, c = divmod(r, NCH)
                ot = g * GN + slot
                tb_sb, dl_sb = tabs[ot]
                qw = dqpool.tile([P, KC], dt.uint8, name="qw", tag="qw", bufs=2)
                nc.sync.dma_start(out=qw, in_=qw_in[ot * P:(ot + 1) * P, c * KC:(c + 1) * KC])
                cw = qw.bitcast(dt.uint32)
                t1w = dqpool.tile([P, KC // 4], dt.uint32, name="t1w", tag="t1w", bufs=2)
                nc.vector.tensor_scalar(out=t1w, in0=cw, scalar1=5, scalar2=0x01010101,
                                        op0=A.logical_shift_right, op1=A.bitwise_and)
                m1 = dqpool.tile([P, KC], dt.int32, name="m1", tag="m1", bufs=1)
                nc.gpsimd.tensor_scalar(out=m1, in0=t1w.bitcast(dt.uint8), scalar1=-1.0,
                                        scalar2=None, op0=A.mult)
                m2 = dqpool.tile([P, KC // 4], dt.uint32, name="m2", tag="m2", bufs=2)
                nc.vector.tensor_scalar(out=m2, in0=cw, scalar1=0x40404040, scalar2=None,
                                        op0=A.bitwise_and)
                m3 = dqpool.tile([P, KC], dt.uint8, name="m3", tag="m3", bufs=2)
                nc.gpsimd.tensor_scalar(out=m3, in0=qw, scalar1=128.0, scalar2=None,
                                        op0=A.is_ge)
                b0 = dqpool.tile([P, KC // 4], dt.uint32, name="b0", tag="b0", bufs=2)
                nc.vector.tensor_scalar(out=b0, in0=cw, scalar1=0x10101010, scalar2=None,
                                        op0=A.bitwise_and)
                zbufs = {0: 2, 1: 1, 2: 2, 3: 1}
                zs = []
                for t in range(4):
                    z = dqpool.tile([P, KC], dt.uint32, name=f"z{t}", tag=f"z{t}", bufs=zbufs[t])
                    nc.vector.tensor_scalar(out=z, in0=m1.bitcast(dt.uint32),
                                            scalar1=dl_sb[:, t:t + 1], scalar2=tb_sb[:, t:t + 1],
                                            op0=A.bitwise_and, op1=A.bitwise_xor)
                    zs.append(z)
                nc.vector.copy_predicated(out=zs[0], mask=m2.bitcast(dt.uint8), data=zs[1])
                nc.vector.copy_predicated(out=zs[2], mask=m2.bitcast(dt.uint8), data=zs[3])
                nc.vector.copy_predicated(out=zs[0], mask=m3, data=zs[2])
                zv = zs[0].bitcast(dt.uint16).rearrange("p (k two) -> p k two", two=2)
                nc.vector.copy_predicated(out=zv[:, :, 0], mask=b0.bitcast(dt.uint8),
                                          data=zv[:, :, 1])
                pending[slot] = (zs[0], c)

            def deq_finish(slot):
                z0, c = pending.pop(slot)
                wt = wtpool.tile([P, KC], dt.bfloat16, name=f"w{slot}_{c}",
                                 tag=f"w{slot}_{c}", bufs=1)
                wch = z0.bitcast(dt.bfloat16).rearrange("p (k two) -> p k two", two=2)[:, :, 0]
                for jg in range(SEG // 4):
                    pt = pst.tile([P, 4 * P], dt.bfloat16, name="pt", tag="pt", bufs=3)
                    for j4 in range(4):
                        j = jg * 4 + j4
                        nc.tensor.transpose(pt[:, j4 * P:(j4 + 1) * P],
                                            wch[:, j * P:(j + 1) * P], ident)
                    nc.scalar.copy(out=wt[:, jg * 4 * P:(jg + 1) * 4 * P], in_=pt)
                wts[(slot, c)] = wt

            def issue_xt(r, sb):
                c = r % NCH
                for j in range(SEG):
                    kt = c * SEG + j
                    xt = xtpool.tile([P, SBW], dt.bfloat16, name="xt", tag="xt", bufs=20)
                    nc.sync.dma_start_transpose(
                        out=xt, in_=xbf_in[sb * SBW:(sb + 1) * SBW, kt * P:(kt + 1) * P])
                    xt_tiles[(sb, j)] = xt

            def mm_pass(r, sb):
                c = r % NCH
                for slot in range(GN):
                    acc = psacc.tile([P, SBW], dt.float32, name="acc", tag="acc", bufs=5)
                    wt = wts[(slot, c)]
                    for j in range(SEG):
                        nc.tensor.matmul(acc, wt[:, j * P:(j + 1) * P], xt_tiles[(sb, j)],
                                         start=(j == 0), stop=(j == SEG - 1))
                    dst = saccs[slot][:, sb * SBW:(sb + 1) * SBW]
                    if c == 0:
                        nc.scalar.copy(out=dst, in_=acc)
                    else:
                        tmp = dqpool.tile([P, SBW], dt.float32, name="tmp", tag="tmp", bufs=4)
                        nc.scalar.copy(out=tmp, in_=acc)
                        nc.gpsimd.tensor_tensor(out=dst, in0=tmp, in1=dst, op=A.add)

            # prologue: round 0 dequant + first xt pass
            for slot in range(GN):
                deq_compute(0, slot)
            for slot in range(GN):
                deq_finish(slot)
            issue_xt(0, 0)

            for r in range(NR):
                g, c = divmod(r, NCH)
                if c == 0:
                    for slot in range(GN):
                        saccs[slot] = saccpool.tile([P, S], dt.float32,
                                                    name=f"sacc{slot}", tag=f"sacc{slot}", bufs=1)
                for sb in range(NSB):
                    if sb < NSB - 1:
                        issue_xt(r, sb + 1)
                    elif r < NR - 1:
                        issue_xt(r + 1, 0)
                    mm_pass(r, sb)
                    if r < NR - 1:
                        deq_compute(r + 1, sb)
                        if sb >= 1:
                            deq_finish(sb - 1)
                if r < NR - 1:
                    deq_finish(3)
                if c == NCH - 1:
                    for slot in range(GN):
                        ot = g * GN + slot
                        nc.sync.dma_start(out=out_d[ot * P:(ot + 1) * P, :], in_=saccs[slot])


_NC_CACHE = None


def _get_nc():
    global _NC_CACHE
    if _NC_CACHE is None:
        nc = bacc.Bacc("TRN2", num_devices=N_CORES)
        _build_kernel(nc)
        nc.compile()
        _NC_CACHE = nc
    return _NC_CACHE


def kernel(x, qweight, lut):
    x = np.asarray(x)
    qweight = np.asarray(qweight)
    lut = np.asarray(lut)
    xbf = np.ascontiguousarray(
        x.reshape(S, K).astype(np.float32, copy=False)).astype(ml_dtypes.bfloat16)

    in_maps = []
    for c in range(N_CORES):
        o0, o1 = c * O, (c + 1) * O
        qb = np.ascontiguousarray(qweight[o0:o1]).view(np.uint8).reshape(O, K)
        tb, dl = _host_tables(lut[o0:o1])
        in_maps.append({"xbf": xbf, "qw": qb, "tb": tb, "dl": dl})

    nc = _get_nc()
    res = bass_utils.run_bass_kernel_spmd(nc, in_maps, core_ids=list(range(N_CORES)))
    out_full = np.concatenate([res.results[c]["out"] for c in range(N_CORES)], axis=0)
    return np.ascontiguousarray(out_full.T).reshape(1, S, 8192).astype(np.float32, copy=False)


# revision 27
# speedup vs baseline: 1.3373x; 1.1111x over previous
"""AnyprecisionLinear (w_bits=4) on 8 TRN2 NeuronCores — self-contained kernel.

kernel(x, qweight, lut) -> out
  x       (1, 2048, 8192) f32
  qweight (8192, 2048)    int32   (4x 8-bit codes per word; idx = code >> 4)
  lut     (8192, 16)      f32     (per-output-row 16-entry table)
  out     (1, 2048, 8192) f32     == einsum('bsk,ok->bso', x, lut[o, idx[o,k]])

Sharding: column-parallel — core i owns output rows [1024*i, 1024*(i+1)).
Each core gets its qweight/lut shard plus the full x (host-cast to bf16),
computes out^T (o, s); the host concatenates the 8 (1024, 2048) shards and
transposes.

Per-core kernel = software-pipelined dequant+matmul:
  - k is split into 4 round-chunks of 2048 per otile; rounds r=0..7 cover
    (group, chunk) pairs for 2 groups of 4 otiles.
  - Dequant (round r+1) is issued interleaved into round r's matmul passes so
    DVE/Pool dequant hides under PE matmul work.
  - Dequant per chunk: byte codes -> per-code full-lane mask m1 (DVE),
    z_t = (m1 & dl_t) ^ tb_t (4x DVE tensor_scalar, bf16-pair tables),
    pair-tree merges via copy_predicated: z2<-z3 on DVE, z0<-z1, z0<-z2 and
    the final u16 lo/hi select on Pool (mask bytes from cheap DVE
    tensor_scalars: &0x40.., &0x80.., &0x10..).
  - Weights are PE-transposed (128x128) into per-(slot, round) Wt tiles.
  - Matmuls accumulate 16-ktile segments per (otile, 512-token sb) in PSUM;
    segments are combined across rounds in SBUF f32 accumulators (Act copy on
    round 0, DVE/Pool scalar_tensor_tensor adds on later rounds).
  - x^T tiles stream via hardware transpose-DMAs from the host-cast bf16 x.
"""
import numpy as np
import ml_dtypes

import concourse.mybir as mybir
from concourse import bacc, bass_utils
from concourse.tile import TileContext
from concourse.masks import make_identity

dt = mybir.dt
A = mybir.AluOpType

O, K, S = 1024, 8192, 2048    # per-core out rows, contraction, tokens
P = 128
KC = 2048                     # codes per dequant chunk (k span per round)
NCH = K // KC                 # rounds (chunk index) per otile = 4
SEG = KC // P                 # ktiles per segment = 16
SBW = 512                     # tokens per psum segment accumulator
NSB = S // SBW                # 4 sb passes per round
GN = 4                        # otiles per group
NG = O // P // GN             # 2 groups
NR = NCH * NG                 # 8 global rounds
N_CORES = 8


def _host_tables(lut_shard):
    U = lut_shard.astype(ml_dtypes.bfloat16).view(np.uint16).astype(np.uint32)
    T = U[:, 0::2] | (U[:, 1::2] << 16)
    tb = T[:, 0::2].copy()
    dl = (T[:, 0::2] ^ T[:, 1::2]).copy()
    return tb, dl


def _build_kernel(nc):
    xbf_in = nc.declare_dram_parameter("xbf", [S, K], dt.bfloat16, isOutput=False)
    qw_in = nc.declare_dram_parameter("qw", [O, K], dt.uint8, isOutput=False)
    tb_in = nc.declare_dram_parameter("tb", [O, 4], dt.uint32, isOutput=False)
    dl_in = nc.declare_dram_parameter("dl", [O, 4], dt.uint32, isOutput=False)
    out_d = nc.declare_dram_parameter("out", [O, S], dt.float32, isOutput=True)

    with TileContext(nc) as tc:
        with tc.tile_pool(name="const", bufs=1) as cpool, \
             tc.tile_pool(name="tabs", bufs=1) as tabpool, \
             tc.tile_pool(name="wt", bufs=1) as wtpool, \
             tc.tile_pool(name="sacc", bufs=1) as saccpool, \
             tc.tile_pool(name="deq", bufs=1) as dqpool, \
             tc.tile_pool(name="xt", bufs=1) as xtpool, \
             tc.tile_pool(name="pst", bufs=1, space="PSUM") as pst, \
             tc.tile_pool(name="psacc", bufs=1, space="PSUM") as psacc:

            ident = cpool.tile([P, P], dt.bfloat16, name="ident")
            idf = cpool.tile([P, P], dt.float32, name="idf")
            make_identity(nc, idf)
            nc.vector.tensor_copy(out=ident, in_=idf)

            NOT = O // P
            tb_all = tabpool.tile([P, 4 * NOT], dt.uint32, name="tb_all")
            dl_all = tabpool.tile([P, 4 * NOT], dt.uint32, name="dl_all")
            nc.sync.dma_start(out=tb_all.rearrange("p (g t) -> p g t", t=4),
                              in_=tb_in.rearrange("(g p) t -> p g t", p=P))
            nc.sync.dma_start(out=dl_all.rearrange("p (g t) -> p g t", t=4),
                              in_=dl_in.rearrange("(g p) t -> p g t", p=P))
            tabs = [(tb_all[:, 4 * ot:4 * (ot + 1)], dl_all[:, 4 * ot:4 * (ot + 1)])
                    for ot in range(NOT)]

            wts = {}          # (slot, c) -> Wt tile [P, KC] (k on partitions)
            pending = {}      # slot -> (z0, c) awaiting transpose
            xt_tiles = {}     # (sb, j) -> xt tile [P, SBW]
            saccs = {}        # slot -> SBUF accumulator [P, S]

            def deq_compute(r, slot):
                g, c = divmod(r, NCH)
                ot = g * GN + slot
                tb_sb, dl_sb = tabs[ot]
                qw = dqpool.tile([P, KC], dt.uint8, name="qw", tag="qw", bufs=2)
                nc.sync.dma_start(out=qw, in_=qw_in[ot * P:(ot + 1) * P, c * KC:(c + 1) * KC])
                cw = qw.bitcast(dt.uint32)
                t1w = dqpool.tile([P, KC // 4], dt.uint32, name="t1w", tag="t1w", bufs=2)
                nc.vector.tensor_scalar(out=t1w, in0=cw, scalar1=5, scalar2=0x01010101,
                                        op0=A.logical_shift_right, op1=A.bitwise_and)
                m1 = dqpool.tile([P, KC], dt.int32, name="m1", tag="m1", bufs=1)
                nc.gpsimd.tensor_scalar(out=m1, in0=t1w.bitcast(dt.uint8), scalar1=-1.0,
                                        scalar2=None, op0=A.mult)
                m2 = dqpool.tile([P, KC // 4], dt.uint32, name="m2", tag="m2", bufs=2)
                nc.vector.tensor_scalar(out=m2, in0=cw, scalar1=0x40404040, scalar2=None,
                                        op0=A.bitwise_and)
                m3 = dqpool.tile([P, KC], dt.uint8, name="m3", tag="m3", bufs=2)
                nc.gpsimd.tensor_scalar(out=m3, in0=qw, scalar1=128.0, scalar2=None,
                                        op0=A.is_ge)
                b0 = dqpool.tile([P, KC // 4], dt.uint32, name="b0", tag="b0", bufs=2)
                nc.vector.tensor_scalar(out=b0, in0=cw, scalar1=0x10101010, scalar2=None,
                                        op0=A.bitwise_and)
                zbufs = {0: 2, 1: 1, 2: 1, 3: 1}
                zs = []
                for t in range(4):
                    z = dqpool.tile([P, KC], dt.uint32, name=f"z{t}", tag=f"z{t}", bufs=zbufs[t])
                    nc.vector.tensor_scalar(out=z, in0=m1.bitcast(dt.uint32),
                                            scalar1=dl_sb[:, t:t + 1], scalar2=tb_sb[:, t:t + 1],
                                            op0=A.bitwise_and, op1=A.bitwise_xor)
                    zs.append(z)
                nc.vector.copy_predicated(out=zs[0], mask=m2.bitcast(dt.uint8), data=zs[1])
                nc.vector.copy_predicated(out=zs[2], mask=m2.bitcast(dt.uint8), data=zs[3])
                nc.vector.copy_predicated(out=zs[0], mask=m3, data=zs[2])
                zv = zs[0].bitcast(dt.uint16).rearrange("p (k two) -> p k two", two=2)
                b0v = b0.bitcast(dt.uint8).rearrange("p (q k) -> p q k", q=4)
                Q = KC // 4
                for q in range(4):
                    nc.vector.copy_predicated(out=zv[:, q * Q:(q + 1) * Q, 0],
                                              mask=b0v[:, q, :],
                                              data=zv[:, q * Q:(q + 1) * Q, 1])
                pending[slot] = (zs[0], c)

            def deq_finish(slot):
                z0, c = pending.pop(slot)
                wt = wtpool.tile([P, KC], dt.bfloat16, name=f"w{slot}_{c}",
                                 tag=f"w{slot}_{c}", bufs=1)
                wch = z0.bitcast(dt.bfloat16).rearrange("p (k two) -> p k two", two=2)[:, :, 0]
                for jg in range(SEG // 4):
                    pt = pst.tile([P, 4 * P], dt.bfloat16, name="pt", tag="pt", bufs=3)
                    for j4 in range(4):
                        j = jg * 4 + j4
                        nc.tensor.transpose(pt[:, j4 * P:(j4 + 1) * P],
                                            wch[:, j * P:(j + 1) * P], ident)
                    nc.scalar.copy(out=wt[:, jg * 4 * P:(jg + 1) * 4 * P], in_=pt)
                wts[(slot, c)] = wt

            def issue_xt(r, sb):
                c = r % NCH
                for j in range(SEG):
                    kt = c * SEG + j
                    xt = xtpool.tile([P, SBW], dt.bfloat16, name="xt", tag="xt", bufs=32)
                    nc.sync.dma_start_transpose(
                        out=xt, in_=xbf_in[sb * SBW:(sb + 1) * SBW, kt * P:(kt + 1) * P])
                    xt_tiles[(sb, j)] = xt

            def mm_pass(r, sb):
                c = r % NCH
                for slot in range(GN):
                    acc = psacc.tile([P, SBW], dt.float32, name="acc", tag="acc", bufs=5)
                    wt = wts[(slot, c)]
                    for j in range(SEG):
                        nc.tensor.matmul(acc, wt[:, j * P:(j + 1) * P], xt_tiles[(sb, j)],
                                         start=(j == 0), stop=(j == SEG - 1))
                    dst = saccs[slot][:, sb * SBW:(sb + 1) * SBW]
                    if c == 0:
                        nc.scalar.copy(out=dst, in_=acc)
                    else:
                        tmp = dqpool.tile([P, SBW], dt.float32, name="tmp", tag="tmp", bufs=2)
                        nc.scalar.copy(out=tmp, in_=acc)
                        nc.gpsimd.tensor_tensor(out=dst, in0=tmp, in1=dst, op=A.add)

            # prologue: round 0 dequant + first two xt passes
            for slot in range(GN):
                deq_compute(0, slot)
            for slot in range(GN):
                deq_finish(slot)
            issue_xt(0, 0)
            issue_xt(0, 1)

            for r in range(NR):
                g, c = divmod(r, NCH)
                if c == 0:
                    for slot in range(GN):
                        saccs[slot] = saccpool.tile([P, S], dt.float32,
                                                    name=f"sacc{slot}", tag=f"sacc{slot}", bufs=1)
                for sb in range(NSB):
                    nxt = r * NSB + sb + 2   # global pass index to prefetch
                    if nxt < NR * NSB:
                        issue_xt(nxt // NSB, nxt % NSB)
                    mm_pass(r, sb)
                    if r < NR - 1:
                        deq_compute(r + 1, sb)
                        if sb >= 1:
                            deq_finish(sb - 1)
                if r < NR - 1:
                    deq_finish(3)
                if c == NCH - 1:
                    for sb in range(NSB):
                        for slot in range(GN):
                            ot = g * GN + slot
                            nc.sync.dma_start(
                                out=out_d[ot * P:(ot + 1) * P, sb * SBW:(sb + 1) * SBW],
                                in_=saccs[slot][:, sb * SBW:(sb + 1) * SBW])


_NC_CACHE = None


def _get_nc():
    global _NC_CACHE
    if _NC_CACHE is None:
        nc = bacc.Bacc("TRN2", num_devices=N_CORES)
        _build_kernel(nc)
        nc.compile()
        _NC_CACHE = nc
    return _NC_CACHE


def kernel(x, qweight, lut):
    x = np.asarray(x)
    qweight = np.asarray(qweight)
    lut = np.asarray(lut)
    xbf = np.ascontiguousarray(
        x.reshape(S, K).astype(np.float32, copy=False)).astype(ml_dtypes.bfloat16)

    in_maps = []
    for c in range(N_CORES):
        o0, o1 = c * O, (c + 1) * O
        qb = np.ascontiguousarray(qweight[o0:o1]).view(np.uint8).reshape(O, K)
        tb, dl = _host_tables(lut[o0:o1])
        in_maps.append({"xbf": xbf, "qw": qb, "tb": tb, "dl": dl})

    nc = _get_nc()
    res = bass_utils.run_bass_kernel_spmd(nc, in_maps, core_ids=list(range(N_CORES)))
    out_full = np.concatenate([res.results[c]["out"] for c in range(N_CORES)], axis=0)
    return np.ascontiguousarray(out_full.T).reshape(1, S, 8192).astype(np.float32, copy=False)


# revision 33
# speedup vs baseline: 1.3545x; 1.0129x over previous
"""AnyprecisionLinear (w_bits=4) on 8 TRN2 NeuronCores — self-contained kernel.

kernel(x, qweight, lut) -> out
  x       (1, 2048, 8192) f32
  qweight (8192, 2048)    int32   (4x 8-bit codes per word; idx = code >> 4)
  lut     (8192, 16)      f32     (per-output-row 16-entry table)
  out     (1, 2048, 8192) f32     == einsum('bsk,ok->bso', x, lut[o, idx[o,k]])

Sharding: column-parallel — core i owns output rows [1024*i, 1024*(i+1)).
Each core gets its qweight/lut shard plus the full x (host-cast to bf16),
computes out^T (o, s); the host concatenates the 8 (1024, 2048) shards and
transposes.

Per-core kernel = software-pipelined dequant+matmul:
  - k is split into 4 round-chunks of 2048 per otile; rounds r=0..7 cover
    (group, chunk) pairs for 2 groups of 4 otiles.
  - Dequant (round r+1) is issued interleaved into round r's matmul passes so
    DVE/Pool dequant hides under PE matmul work.
  - Dequant per chunk: byte codes -> per-code full-lane mask m1 (DVE),
    z_t = (m1 & dl_t) ^ tb_t (4x DVE tensor_scalar, bf16-pair tables),
    pair-tree merges via copy_predicated: z2<-z3 on DVE, z0<-z1, z0<-z2 and
    the final u16 lo/hi select on Pool (mask bytes from cheap DVE
    tensor_scalars: &0x40.., &0x80.., &0x10..).
  - Weights are PE-transposed (128x128) into per-(slot, round) Wt tiles.
  - Matmuls accumulate 16-ktile segments per (otile, 512-token sb) in PSUM;
    segments are combined across rounds in SBUF f32 accumulators (Act copy on
    round 0, DVE/Pool scalar_tensor_tensor adds on later rounds).
  - x^T tiles stream via hardware transpose-DMAs from the host-cast bf16 x.
"""
import numpy as np
import ml_dtypes

import concourse.mybir as mybir
from concourse import bacc, bass_utils
from concourse.tile import TileContext
from concourse.masks import make_identity

dt = mybir.dt
A = mybir.AluOpType

O, K, S = 1024, 8192, 2048    # per-core out rows, contraction, tokens
P = 128
KC = 2048                     # codes per dequant chunk (k span per round)
NCH = K // KC                 # rounds (chunk index) per otile = 4
SEG = KC // P                 # ktiles per segment = 16
SBW = 512                     # tokens per psum segment accumulator
NSB = S // SBW                # 4 sb passes per round
GN = 4                        # otiles per group
NG = O // P // GN             # 2 groups
NR = NCH * NG                 # 8 global rounds
N_CORES = 8


def _host_tables(lut_shard):
    U = lut_shard.astype(ml_dtypes.bfloat16).view(np.uint16).astype(np.uint32)
    T = U[:, 0::2] | (U[:, 1::2] << 16)
    tb = T[:, 0::2].copy()
    dl = (T[:, 0::2] ^ T[:, 1::2]).copy()
    return tb, dl


def _build_kernel(nc):
    xbf_in = nc.declare_dram_parameter("xbf", [S, K], dt.bfloat16, isOutput=False)
    qw_in = nc.declare_dram_parameter("qw", [O, K], dt.uint8, isOutput=False)
    tb_in = nc.declare_dram_parameter("tb", [O, 4], dt.uint32, isOutput=False)
    dl_in = nc.declare_dram_parameter("dl", [O, 4], dt.uint32, isOutput=False)
    out_d = nc.declare_dram_parameter("out", [O, S], dt.float32, isOutput=True)

    with TileContext(nc) as tc:
        with tc.tile_pool(name="const", bufs=1) as cpool, \
             tc.tile_pool(name="tabs", bufs=1) as tabpool, \
             tc.tile_pool(name="wt", bufs=1) as wtpool, \
             tc.tile_pool(name="sacc", bufs=1) as saccpool, \
             tc.tile_pool(name="deq", bufs=1) as dqpool, \
             tc.tile_pool(name="xt", bufs=1) as xtpool, \
             tc.tile_pool(name="pst", bufs=1, space="PSUM") as pst, \
             tc.tile_pool(name="psacc", bufs=1, space="PSUM") as psacc:

            ident = cpool.tile([P, P], dt.bfloat16, name="ident")
            idf = cpool.tile([P, P], dt.float32, name="idf")
            make_identity(nc, idf)
            nc.vector.tensor_copy(out=ident, in_=idf)

            NOT = O // P
            dqpool_early = dqpool  # alias for clarity: qw0 prefetch precedes tabs
            qw0_pre = dqpool_early.tile([P, KC], dt.uint8, name="qw", tag="qw", bufs=2)
            nc.sync.dma_start(out=qw0_pre, in_=qw_in[0:P, 0:KC])

            tb_all = tabpool.tile([P, 4 * NOT], dt.uint32, name="tb_all")
            dl_all = tabpool.tile([P, 4 * NOT], dt.uint32, name="dl_all")
            nc.sync.dma_start(out=tb_all.rearrange("p (g t) -> p g t", t=4),
                              in_=tb_in.rearrange("(g p) t -> p g t", p=P))
            nc.sync.dma_start(out=dl_all.rearrange("p (g t) -> p g t", t=4),
                              in_=dl_in.rearrange("(g p) t -> p g t", p=P))
            tabs = [(tb_all[:, 4 * ot:4 * (ot + 1)], dl_all[:, 4 * ot:4 * (ot + 1)])
                    for ot in range(NOT)]

            wts = {}          # (slot, c) -> Wt tile [P, KC] (k on partitions)
            pending = {}      # slot -> (z0, c) awaiting transpose
            xt_tiles = {}     # (sb, j) -> xt tile [P, SBW]
            saccs = {}        # slot -> SBUF accumulator [P, S]

            def deq_compute(r, slot, nsplit=1, pre_qw=None):
                g, c = divmod(r, NCH)
                ot = g * GN + slot
                tb_sb, dl_sb = tabs[ot]
                if pre_qw is not None:
                    qw = pre_qw
                else:
                    qw = dqpool.tile([P, KC], dt.uint8, name="qw", tag="qw", bufs=2)
                    nc.sync.dma_start(out=qw,
                                      in_=qw_in[ot * P:(ot + 1) * P, c * KC:(c + 1) * KC])
                t1w = dqpool.tile([P, KC // 4], dt.uint32, name="t1w", tag="t1w", bufs=2)
                m1 = dqpool.tile([P, KC], dt.int32, name="m1", tag="m1", bufs=1)
                m2 = dqpool.tile([P, KC // 4], dt.uint32, name="m2", tag="m2", bufs=2)
                m3 = dqpool.tile([P, KC], dt.uint8, name="m3", tag="m3", bufs=2)
                b0 = dqpool.tile([P, KC // 4], dt.uint32, name="b0", tag="b0", bufs=2)
                zbufs = {0: 2, 1: 1, 2: 1, 3: 1}
                zs = [dqpool.tile([P, KC], dt.uint32, name=f"z{t}", tag=f"z{t}",
                                  bufs=zbufs[t]) for t in range(4)]
                L = KC // nsplit
                W = L // 4
                for s in range(nsplit):
                    kb, wb = s * L, s * W
                    qs = qw[:, kb:kb + L]
                    cws = qs.bitcast(dt.uint32)
                    t1s = t1w[:, wb:wb + W]
                    nc.vector.tensor_scalar(out=t1s, in0=cws, scalar1=5, scalar2=0x01010101,
                                            op0=A.logical_shift_right, op1=A.bitwise_and)
                    m1s = m1[:, kb:kb + L]
                    nc.gpsimd.tensor_scalar(out=m1s, in0=t1s.bitcast(dt.uint8), scalar1=-1.0,
                                            scalar2=None, op0=A.mult)
                    m2s = m2[:, wb:wb + W]
                    nc.vector.tensor_scalar(out=m2s, in0=cws, scalar1=0x40404040, scalar2=None,
                                            op0=A.bitwise_and)
                    m3s = m3[:, kb:kb + L]
                    nc.gpsimd.tensor_scalar(out=m3s, in0=qs, scalar1=128.0, scalar2=None,
                                            op0=A.is_ge)
                    b0s = b0[:, wb:wb + W]
                    nc.vector.tensor_scalar(out=b0s, in0=cws, scalar1=0x10101010, scalar2=None,
                                            op0=A.bitwise_and)
                    for t in range(4):
                        nc.vector.tensor_scalar(out=zs[t][:, kb:kb + L],
                                                in0=m1s.bitcast(dt.uint32),
                                                scalar1=dl_sb[:, t:t + 1],
                                                scalar2=tb_sb[:, t:t + 1],
                                                op0=A.bitwise_and, op1=A.bitwise_xor)
                    z0s, z1s = zs[0][:, kb:kb + L], zs[1][:, kb:kb + L]
                    z2s, z3s = zs[2][:, kb:kb + L], zs[3][:, kb:kb + L]
                    nc.vector.copy_predicated(out=z0s, mask=m2s.bitcast(dt.uint8), data=z1s)
                    nc.vector.copy_predicated(out=z2s, mask=m2s.bitcast(dt.uint8), data=z3s)
                    nc.vector.copy_predicated(out=z0s, mask=m3s, data=z2s)
                    zv = z0s.bitcast(dt.uint16).rearrange("p (k two) -> p k two", two=2)
                    b0v = b0s.bitcast(dt.uint8).rearrange("p (q k) -> p q k", q=4 // nsplit)
                    Q = L // (4 // nsplit)
                    for q in range(4 // nsplit):
                        nc.vector.copy_predicated(out=zv[:, q * Q:(q + 1) * Q, 0],
                                                  mask=b0v[:, q, :],
                                                  data=zv[:, q * Q:(q + 1) * Q, 1])
                pending[slot] = (zs[0], c)

            def deq_finish(slot):
                z0, c = pending.pop(slot)
                wt = wtpool.tile([P, KC], dt.bfloat16, name=f"w{slot}_{c}",
                                 tag=f"w{slot}_{c}", bufs=1)
                wch = z0.bitcast(dt.bfloat16).rearrange("p (k two) -> p k two", two=2)[:, :, 0]
                for jg in range(SEG // 4):
                    pt = pst.tile([P, 4 * P], dt.bfloat16, name="pt", tag="pt", bufs=3)
                    for j4 in range(4):
                        j = jg * 4 + j4
                        nc.tensor.transpose(pt[:, j4 * P:(j4 + 1) * P],
                                            wch[:, j * P:(j + 1) * P], ident)
                    nc.scalar.copy(out=wt[:, jg * 4 * P:(jg + 1) * 4 * P], in_=pt)
                wts[(slot, c)] = wt

            def issue_xt(r, sb):
                c = r % NCH
                for j in range(SEG):
                    kt = c * SEG + j
                    xt = xtpool.tile([P, SBW], dt.bfloat16, name="xt", tag="xt", bufs=32)
                    nc.sync.dma_start_transpose(
                        out=xt, in_=xbf_in[sb * SBW:(sb + 1) * SBW, kt * P:(kt + 1) * P])
                    xt_tiles[(sb, j)] = xt

            def mm_block(r, sb, slot):
                c = r % NCH
                acc = psacc.tile([P, SBW], dt.float32, name="acc", tag="acc", bufs=5)
                wt = wts[(slot, c)]
                for j in range(SEG):
                    nc.tensor.matmul(acc, wt[:, j * P:(j + 1) * P], xt_tiles[(sb, j)],
                                     start=(j == 0), stop=(j == SEG - 1))
                dst = saccs[slot][:, sb * SBW:(sb + 1) * SBW]
                if c == 0:
                    nc.scalar.copy(out=dst, in_=acc)
                else:
                    tmp = dqpool.tile([P, SBW], dt.float32, name="tmp", tag="tmp", bufs=2)
                    nc.scalar.copy(out=tmp, in_=acc)
                    nc.gpsimd.tensor_tensor(out=dst, in0=tmp, in1=dst, op=A.add)

            def mm_pass(r, sb):
                for slot in range(GN):
                    mm_block(r, sb, slot)

            # prologue: round 0 dequant; diagonal mm issue so PE follows the
            # DVE chunk stream slot-by-slot instead of waiting for all four.
            for slot in range(GN):
                deq_compute(0, slot, nsplit=2 if slot == 0 else 1,
                            pre_qw=qw0_pre if slot == 0 else None)
            issue_xt(0, 0)
            issue_xt(0, 1)
            for slot in range(GN):
                saccs[slot] = saccpool.tile([P, S], dt.float32,
                                            name=f"sacc{slot}", tag=f"sacc{slot}", bufs=1)
            for slot in range(GN):
                deq_finish(slot)
                mm_block(0, 0, slot)
                mm_block(0, 1, slot)

            for r in range(NR):
                g, c = divmod(r, NCH)
                if c == 0 and r > 0:
                    for slot in range(GN):
                        saccs[slot] = saccpool.tile([P, S], dt.float32,
                                                    name=f"sacc{slot}", tag=f"sacc{slot}", bufs=1)
                for sb in range(NSB):
                    nxt = r * NSB + sb + 2   # global pass index to prefetch
                    if nxt < NR * NSB:
                        issue_xt(nxt // NSB, nxt % NSB)
                    if not (r == 0 and sb < 2):
                        # passes (0,0)/(0,1) were issued in the diagonal prologue
                        mm_pass(r, sb)
                    if r < NR - 1:
                        deq_compute(r + 1, sb)
                        if sb >= 1:
                            deq_finish(sb - 1)
                if r < NR - 1:
                    deq_finish(3)
                if c == NCH - 1:
                    for sb in range(NSB):
                        for slot in range(GN):
                            ot = g * GN + slot
                            nc.sync.dma_start(
                                out=out_d[ot * P:(ot + 1) * P, sb * SBW:(sb + 1) * SBW],
                                in_=saccs[slot][:, sb * SBW:(sb + 1) * SBW])


_NC_CACHE = None


def _get_nc():
    global _NC_CACHE
    if _NC_CACHE is None:
        nc = bacc.Bacc("TRN2", num_devices=N_CORES)
        _build_kernel(nc)
        nc.compile()
        _NC_CACHE = nc
    return _NC_CACHE


def kernel(x, qweight, lut):
    x = np.asarray(x)
    qweight = np.asarray(qweight)
    lut = np.asarray(lut)
    xbf = np.ascontiguousarray(
        x.reshape(S, K).astype(np.float32, copy=False)).astype(ml_dtypes.bfloat16)

    in_maps = []
    for c in range(N_CORES):
        o0, o1 = c * O, (c + 1) * O
        qb = np.ascontiguousarray(qweight[o0:o1]).view(np.uint8).reshape(O, K)
        tb, dl = _host_tables(lut[o0:o1])
        in_maps.append({"xbf": xbf, "qw": qb, "tb": tb, "dl": dl})

    nc = _get_nc()
    res = bass_utils.run_bass_kernel_spmd(nc, in_maps, core_ids=list(range(N_CORES)))
    out_full = np.concatenate([res.results[c]["out"] for c in range(N_CORES)], axis=0)
    return np.ascontiguousarray(out_full.T).reshape(1, S, 8192).astype(np.float32, copy=False)
